# revision 20
# baseline (speedup 1.0000x reference)
"""Trainium2 Bass kernel for the LDE1D vq_codebook problem, v8.

v7 -> v8: restructured for HW LDWEIGHTS cost (unmodeled by CoreSim's
cost model; ldweights ~= stationary_cols/1.2 ns, serialized with the
matmul stream unless hidden). Changes:

- acc consolidated: 3 matmuls/tile (x0^T w, x1^T w, ones^T w) -> ONE
  w-stationary matmul out[k, 0:257] = w^T @ [x | 1], with the ones
  column baked into the host-side x layout (D padded 256->257). nacc is
  column 256. Saves 2 LDWEIGHTS + 2 matmul dispatch floors per tile.
  The acc output lands directly in [k, d] layout (no host transpose).
- group size G=8 (amortizes the lnu bias matmul + instr overheads);
  pt/pq/acc PSUM pools sized to exactly 8 banks.
- softmax chain engine re-balance: exp on ACT, dt/recip/scl on DVE,
  w-scale on Pool (Pool cannot read PSUM or run divide on HW);
  PSUM->SBUF x^T copy split DVE/ACT by column range.

Math identical to v7 (see v7 docstring); host epilogue
e[k,d] = acc[k,d] / nacc[k] - mu[k,d].
"""

import sys
from contextlib import ExitStack

import numpy as np

sys.path.insert(0, "/opt/trn_rl_repo")

import ml_dtypes

import concourse.bass as bass
import concourse.tile as tile
from concourse import bacc, mybir
from concourse.bass_utils import run_bass_kernel_spmd

BF16 = mybir.dt.bfloat16
F32 = mybir.dt.float32

B, T, D, K = 64, 4096, 256, 64
DP = D + 1                  # x padded with a ones column (nacc fold)
NCORES = 8
BPC = B // NCORES
TT = 128
G = 8                       # tiles per group (oct)
CSPL1 = 128                 # x^T copy split: DVE [0:c1], ACT [c1:256]

OFF_A, OFF_B, OFF_C, OFF_D, OFF_U, OFF_E, OFF_F, OFF_G = 0, 1, 2, 3, 4, 5, 6, 7
DRAIN = OFF_G + 1


def build_program(bpc=BPC, t=T, reps=1, trn_type="TRN2"):
    ntiles = t // TT
    assert ntiles % G == 0
    nc = bacc.Bacc(trn_type, target_bir_lowering=False, debug=False,
                   num_devices=NCORES)
    x_d = nc.dram_tensor("x", [bpc, TT, ntiles, DP], BF16,
                         kind="ExternalInput").ap()
    wsT_d = nc.dram_tensor("wsT", [TT, bpc * ntiles], F32,
                           kind="ExternalInput").ap()
    muT2_d = nc.dram_tensor("muT2", [128, 2 * K], BF16,
                            kind="ExternalInput").ap()
    lnu_d = nc.dram_tensor("lnu", [TT, G, K], BF16,
                           kind="ExternalInput").ap()
    ident_d = nc.dram_tensor("ident", [128, 128], BF16,
                             kind="ExternalInput").ap()
    accT_d = nc.dram_tensor("accT", [bpc, K, DP], F32,
                            kind="ExternalOutput").ap()

    with tile.TileContext(nc) as tc, ExitStack() as ctx:
        _body(ctx, tc, accT_d, x_d, wsT_d, muT2_d, lnu_d, ident_d,
              bpc, ntiles, reps)
    nc.compile()
    return nc


def _body(ctx, tc, accT_d, x_d, wsT_d, muT2_d, lnu_d, ident_d,
          bpc, ntiles, reps):
    nc = tc.nc
    ngroups = ntiles // G
    xb_pool = ctx.enter_context(tc.tile_pool(name="xb", bufs=3))
    const = ctx.enter_context(tc.tile_pool(name="const", bufs=1))
    muT2 = const.tile([128, 2 * K], BF16)
    urep = const.tile([TT, G, K], BF16)
    ident = const.tile([128, 128], BF16)
    wsall = const.tile([TT, bpc * ntiles], F32)
    # batch 0's x sub-DMAs interleaved with const loads so the pipeline
    # fills as early as possible (SP queue is in-order)
    nsplit0 = min(8, ntiles)
    q0 = ntiles // nsplit0
    xbt0 = xb_pool.tile([TT, ntiles, DP], BF16, name="xbt0")

    def _x0(hh):
        nc.sync.dma_start(xbt0[:, hh * q0:(hh + 1) * q0, :],
                          x_d[0][:, hh * q0:(hh + 1) * q0, :])

    _x0(0)
    nc.sync.dma_start(ident[:], ident_d[:])
    if nsplit0 > 1:
        _x0(1)
    nc.sync.dma_start(muT2[:], muT2_d[:])
    nc.sync.dma_start(urep[:], lnu_d[:])
    if nsplit0 > 2:
        _x0(2)
    nc.sync.dma_start(wsall[:], wsT_d[:])
    for hh in range(3, nsplit0):
        _x0(hh)

    xt_pool = ctx.enter_context(tc.tile_pool(name="xt", bufs=2))
    p_pool = ctx.enter_context(tc.tile_pool(name="p", bufs=2))
    pu_pool = ctx.enter_context(tc.tile_pool(name="pu", bufs=2))
    w_pool = ctx.enter_context(tc.tile_pool(name="w", bufs=2))
    dt_pool = ctx.enter_context(tc.tile_pool(name="dt", bufs=2))
    rd_pool = ctx.enter_context(tc.tile_pool(name="rd", bufs=2))
    scl_pool = ctx.enter_context(tc.tile_pool(name="scl", bufs=2))
    res_pool = ctx.enter_context(tc.tile_pool(name="res", bufs=2))
    pt_psum = ctx.enter_context(tc.tile_pool(name="pt", bufs=2, space="PSUM"))
    pq_psum = ctx.enter_context(tc.tile_pool(name="pq", bufs=2, space="PSUM"))
    pa_psum = ctx.enter_context(tc.tile_pool(name="pa", bufs=2, space="PSUM"))

    nbat = reps * bpc
    ntotg = nbat * ngroups
    xb = {0: xbt0}
    pt_t, xt_t, pq_t, p_t, pu_t, w_t, scl_t = {}, {}, {}, {}, {}, {}, {}
    accb = {}
    next_nb = 1

    for it in range(ntotg + DRAIN):
        while next_nb < nbat and next_nb * ngroups <= it + 5:
            xbt = xb_pool.tile([TT, ntiles, DP], BF16)
            nsplit = min(8, ntiles)
            q4 = ntiles // nsplit
            for hh in range(nsplit):
                nc.sync.dma_start(
                    xbt[:, hh * q4:(hh + 1) * q4, :],
                    x_d[next_nb % bpc][:, hh * q4:(hh + 1) * q4, :])
            xb[next_nb] = xbt
            next_nb += 1

        gg = it - OFF_A
        if 0 <= gg < ntotg:  # A: PE transposes (16 per oct)
            nb, g = gg // ngroups, gg % ngroups
            pt = pt_psum.tile([128, G, 256], BF16)
            for j in range(G):
                xin = xb[nb][:, g * G + j, :]
                nc.tensor.transpose(pt[:, j, 0:128], xin[:, 0:128], ident[:])
                nc.tensor.transpose(pt[:, j, 128:256], xin[:, 128:256],
                                    ident[:])
            pt_t[gg] = pt

        gg = it - OFF_B
        if 0 <= gg < ntotg:  # B: copy PSUM->SBUF split DVE/ACT
            pt = pt_t.pop(gg)
            xt = xt_pool.tile([128, G, 256], BF16)
            nc.vector.tensor_copy(xt[:, :, 0:CSPL1], pt[:, :, 0:CSPL1])
            nc.scalar.copy(xt[:, :, CSPL1:256], pt[:, :, CSPL1:256])
            xt_t[gg] = xt

        gg = it - OFF_C
        if 0 <= gg < ntotg:  # C: PE q matmuls, one PSUM group
            xt = xt_t.pop(gg)
            pq = pq_psum.tile([TT, G, K], F32)
            for j in range(G):
                nc.tensor.matmul(pq[:, j, :], xt[:, j, 0:128], muT2[:, 0:K],
                                 start=(j == 0), stop=False)
                nc.tensor.matmul(pq[:, j, :], xt[:, j, 128:256],
                                 muT2[:, K:2 * K], start=False,
                                 stop=(j == G - 1))
            pq_t[gg] = pq

        gg = it - OFF_D
        if 0 <= gg < ntotg:  # D: ACT exp
            p = p_pool.tile([TT, G, K], BF16)
            nc.scalar.activation(p[:], pq_t.pop(gg)[:],
                                 mybir.ActivationFunctionType.Exp)
            p_t[gg] = p

        gg = it - OFF_U
        if 0 <= gg < ntotg:  # U: Pool pu = p * u (u replicated per-partition)
            pu = pu_pool.tile([TT, G, K], BF16)
            nc.gpsimd.tensor_tensor(pu[:], p_t.pop(gg)[:], urep[:],
                                    mybir.AluOpType.mult)
            pu_t[gg] = pu

        gg = it - OFF_E
        if 0 <= gg < ntotg:  # E: DVE dt, rd, scl
            nb, g = gg // ngroups, gg % ngroups
            pu = pu_t[gg]
            dt = dt_pool.tile([TT, G], F32)
            nc.vector.tensor_reduce(dt[:], pu[:], mybir.AxisListType.X,
                                    mybir.AluOpType.add)
            rd = rd_pool.tile([TT, G], F32)
            nc.vector.reciprocal(rd[:], dt[:])
            scl = scl_pool.tile([TT, G, 1], F32)
            col = (nb % bpc) * ntiles + g * G
            nc.vector.tensor_tensor(
                scl[:, :, 0], wsall[:, col:col + G], rd[:],
                mybir.AluOpType.mult)
            scl_t[gg] = scl

        gg = it - OFF_F
        if 0 <= gg < ntotg:  # F: Pool w = pu * scl (broadcast over k)
            pu = pu_t.pop(gg)
            scl = scl_t.pop(gg)
            w = w_pool.tile([TT, G, K], BF16)
            sb, wb = bass.broadcast_tensor_aps(scl[:], w[:])
            nc.gpsimd.tensor_tensor(w[:], pu[:], sb, mybir.AluOpType.mult)
            w_t[gg] = w

        gg = it - OFF_G
        if 0 <= gg < ntotg:  # G: PE acc matmuls, col-tiled even/odd pairs
            nb, g = gg // ngroups, gg % ngroups
            if g == 0:
                accb[nb] = pa_psum.tile([K, DP], F32, name="accb")
            ab = accb[nb]
            w = w_t.pop(gg)
            for j in range(G):
                ti = g * G + j
                first = ti == 0
                last = ti == ntiles - 1
                nc.tensor.matmul(ab[:], w[:, j, :], xb[nb][:, ti, :],
                                 start=first, stop=last,
                                 skip_group_check=True)
            if g == ngroups - 1:
                b = nb % bpc
                accs = res_pool.tile([K, DP], F32, tag="accs")
                nc.vector.tensor_copy(accs[:, 0:144], ab[:, 0:144])
                nc.scalar.copy(accs[:, 144:DP], ab[:, 144:DP])
                nc.sync.dma_start(accT_d[b], accs[:])
                del accb[nb], xb[nb]


def make_inputs(x, weights, mu, s, bpc=BPC, t=T):
    """Host-side prep: shard + precompute small replicated tensors."""
    ntiles = t // TT
    s = np.asarray(s, dtype=np.float32)
    s0 = float(s[0])
    if not np.allclose(s, s0):
        raise NotImplementedError("kernel assumes uniform s (as in setup)")
    mu = np.ascontiguousarray(mu, dtype=np.float32)
    mu2t = (2.0 * s0 * mu).T.astype(ml_dtypes.bfloat16)      # [D, K]
    muT2 = np.concatenate([mu2t[:128], mu2t[128:]], axis=1)  # [128, 2K]
    c = s0 * np.sum(mu.astype(np.float64) ** 2, axis=1)
    u = np.exp(-c).astype(np.float32)
    urep = np.broadcast_to(np.tile(u, G).reshape(1, G, K),
                           (TT, G, K)).astype(ml_dtypes.bfloat16)
    ident = np.eye(128, dtype=ml_dtypes.bfloat16)
    ncores = x.shape[0] // bpc
    # one fused pass over x: cast fp32->bf16 + tile-permute into the
    # DMA layout [B, TT, ntiles, D+1] with a baked-in ones column
    xbf = np.empty((x.shape[0], TT, ntiles, DP), dtype=ml_dtypes.bfloat16)
    xbf[..., 0:D] = x[:, :t].reshape(x.shape[0], ntiles, TT, D).transpose(
        0, 2, 1, 3)
    xbf[..., D] = 1.0
    ws = np.asarray(weights[:, :t], dtype=np.float32)
    wsT = ws.reshape(x.shape[0], ntiles, TT).transpose(2, 0, 1)
    in_maps = []
    for ci in range(ncores):
        sl = slice(ci * bpc, (ci + 1) * bpc)
        in_maps.append({
            "x": xbf[sl],
            "wsT": np.ascontiguousarray(
                wsT[:, sl].reshape(TT, bpc * ntiles)),
            "muT2": muT2, "lnu": urep, "ident": ident,
        })
    return in_maps


OUTPUT_NAMES = ["accT"]


def postprocess(outs, mu, bpc=BPC):
    S = outs["accT"]                            # [bpc, K, 257]
    e = S[:, :, 0:D] / S[:, :, D:DP] - mu[None]
    return e.reshape(bpc, K * D).astype(np.float32)


_CACHE = {}


def _get_program():
    if "nc" not in _CACHE:
        _CACHE["nc"] = build_program()
    return _CACHE["nc"]


def kernel(x, weights, mu, s):
    x = np.asarray(x)
    weights = np.asarray(weights)
    mu = np.asarray(mu, dtype=np.float32)
    s = np.asarray(s, dtype=np.float32)
    nc = _get_program()
    in_maps = make_inputs(x, weights, mu, s)
    res = run_bass_kernel_spmd(nc, in_maps, core_ids=list(range(NCORES)))
    outs = [postprocess(res.results[ci], mu) for ci in range(NCORES)]
    return np.concatenate(outs, axis=0)


if __name__ == "__main__":
    rng = np.random.default_rng(0)
    x = rng.standard_normal((B, T, D), dtype=np.float32)
    w = rng.random((B, T), dtype=np.float32)
    mu = (0.1 * rng.standard_normal((K, D))).astype(np.float32)
    s = np.ones((K,), dtype=np.float32)
    out = kernel(x, weights=w, mu=mu, s=s)
    print("out", out.shape, out.dtype)
